# revision 1
# baseline (speedup 1.0000x reference)
"""ContextBasedLinear Trainium2 kernel.

Computes out = mu * x + gamma * sum(x, axis=1, keepdims=True) for
x: [64, 1024, 512] f32, mu/gamma: [1] f32.

Sharding: data-parallel on the batch dim — 8 batches per core on 8
NeuronCores; mu/gamma replicated. No cross-core comms needed.

Per-core program (x_c: [8, 1024, 512]):
  for each batch b:
    - DMA x_c[b] ([1024, 512]) into SBUF as [128, 4096]: partition p
      holds set rows 8p..8p+7 (16 KB contiguous per partition).
    - colsum: psum_s[1, 512] += ones[128,1].T @ xt[:, r*512:(r+1)*512]
      for r in 0..7 (PE reduces the partition dim; PSUM accumulates
      the within-partition r dim).
    - psum_b[128, 512] = (gamma*ones)[1,128].T @ s[1,512]  (rank-1
      broadcast of gamma*colsum to all partitions).
    - out = (x * mu) + psum_b  in ONE fused DVE scalar_tensor_tensor
      pass, with psum_b read through a step-0 broadcast AP over r.
    - DMA out tile back.
"""

import numpy as np

import concourse.bacc as bacc
import concourse.mybir as mybir
import concourse.tile as tile
from concourse.bass_utils import run_bass_kernel_spmd

N_CORES = 8
B_FULL = 64
B_PER = B_FULL // N_CORES  # 8 batches per core
N_SET = 1024
D = 512
P = 128
R = N_SET // P  # 8 set-rows per partition
F = R * D  # 4096 free elems per partition

_cache = {}


def build_nc():
    if "nc" in _cache:
        return _cache["nc"]
    f32 = mybir.dt.float32
    nc = bacc.Bacc(
        "TRN2", target_bir_lowering=False, debug=False, num_devices=N_CORES
    )
    x_d = nc.dram_tensor("x", [B_PER, N_SET, D], f32, kind="ExternalInput").ap()
    mu_d = nc.dram_tensor("mu", [1], f32, kind="ExternalInput").ap()
    gamma_d = nc.dram_tensor("gamma", [1], f32, kind="ExternalInput").ap()
    out_d = nc.dram_tensor("out", [B_PER, N_SET, D], f32, kind="ExternalOutput").ap()

    with tile.TileContext(nc) as tc:
        with (
            tc.tile_pool(name="consts", bufs=1) as consts,
            tc.tile_pool(name="xp", bufs=3) as xp,
            tc.tile_pool(name="op", bufs=3) as op,
            tc.tile_pool(name="sp", bufs=2) as sp,
            tc.tile_pool(name="ps", bufs=2, space="PSUM") as ps,
            tc.tile_pool(name="pb", bufs=2, space="PSUM") as pb,
        ):
            # ---- constants ----
            ones_col = consts.tile([P, 1], f32)  # colsum lhsT (K=128, M=1)
            nc.vector.memset(ones_col, 1.0)
            ones_row = consts.tile([1, P], f32)  # bcast lhsT template
            nc.vector.memset(ones_row, 1.0)
            mu_sb = consts.tile([1, 1], f32)
            nc.sync.dma_start(mu_sb, mu_d[None, :])
            gamma_sb = consts.tile([1, 1], f32)
            nc.sync.dma_start(gamma_sb, gamma_d[None, :])
            # gamma_row[1,128] = gamma * ones (runtime scalar from SBUF)
            gamma_row = consts.tile([1, P], f32)
            nc.vector.tensor_scalar_mul(gamma_row, ones_row, gamma_sb[:])
            # mu replicated to all 128 partitions via rank-1 PE matmul
            psum_mu = ps.tile([P, 1], f32, tag="psmu")
            nc.tensor.matmul(psum_mu, lhsT=ones_row[:], rhs=mu_sb[:], start=True, stop=True)
            mu_col = consts.tile([P, 1], f32)
            nc.vector.tensor_copy(mu_col, psum_mu)

            # ---- per-batch pipeline ----
            for b in range(B_PER):
                x_view = x_d[b].rearrange("(p r) d -> p (r d)", p=P)
                o_view = out_d[b].rearrange("(p r) d -> p (r d)", p=P)

                xt = xp.tile([P, F], f32, tag="xt")
                nc.sync.dma_start(xt, x_view)

                # colsum over all 1024 set rows -> psum_s[1, 512]
                psum_s = ps.tile([1, D], f32, tag="pss")
                for r in range(R):
                    nc.tensor.matmul(
                        psum_s,
                        lhsT=ones_col[:],
                        rhs=xt[:, r * D : (r + 1) * D],
                        start=(r == 0),
                        stop=(r == R - 1),
                    )
                s_sb = sp.tile([1, D], f32, tag="ssb")
                nc.scalar.copy(s_sb, psum_s)

                # broadcast gamma*colsum to [128, 512] via rank-1 matmul
                psum_b = pb.tile([P, D], f32, tag="psb")
                nc.tensor.matmul(
                    psum_b, lhsT=gamma_row[:], rhs=s_sb[:], start=True, stop=True
                )

                # fused: out = (x * mu) + bcast   (single DVE pass)
                ot = op.tile([P, F], f32, tag="ot")
                nc.vector.scalar_tensor_tensor(
                    out=ot[:].rearrange("p (r d) -> p r d", r=R),
                    in0=xt[:].rearrange("p (r d) -> p r d", r=R),
                    scalar=mu_col[:],
                    in1=psum_b[:, None, :].broadcast_to([P, R, D]),
                    op0=mybir.AluOpType.mult,
                    op1=mybir.AluOpType.add,
                )
                nc.sync.dma_start(o_view, ot)

    nc.compile()
    _cache["nc"] = nc
    return nc


def run_spmd(x, mu, gamma, **spmd_kwargs):
    nc = build_nc()
    x = np.ascontiguousarray(x, dtype=np.float32)
    mu = np.ascontiguousarray(mu, dtype=np.float32)
    gamma = np.ascontiguousarray(gamma, dtype=np.float32)
    in_maps = [
        {"x": x[c * B_PER : (c + 1) * B_PER], "mu": mu, "gamma": gamma}
        for c in range(N_CORES)
    ]
    return run_bass_kernel_spmd(nc, in_maps, list(range(N_CORES)), **spmd_kwargs)


def kernel(x, mu, gamma):
    res = run_spmd(x, mu, gamma)
    out = np.concatenate([r["out"] for r in res.results], axis=0)
    return out


# revision 3
# speedup vs baseline: 1.1537x; 1.1537x over previous
"""ContextBasedLinear Trainium2 kernel.

Computes out = mu * x + gamma * sum(x, axis=1, keepdims=True) for
x: [64, 1024, 512] f32, mu/gamma: [1] f32.

Sharding: data-parallel on the batch dim — 8 batches per core on 8
NeuronCores; mu/gamma replicated. No cross-core comms needed.

Per-core program (x_c: [8, 1024, 512]):
  for each batch b:
    - DMA x_c[b] ([1024, 512]) into SBUF as [128, 4096]: partition p
      holds set rows 8p..8p+7 (16 KB contiguous per partition).
    - colsum: psum_s[1, 512] += ones[128,1].T @ xt[:, r*512:(r+1)*512]
      for r in 0..7 (PE reduces the partition dim; PSUM accumulates
      the within-partition r dim).
    - psum_b[128, 512] = (gamma*ones)[1,128].T @ s[1,512]  (rank-1
      broadcast of gamma*colsum to all partitions).
    - out = (x * mu) + psum_b  in ONE fused DVE scalar_tensor_tensor
      pass, with psum_b read through a step-0 broadcast AP over r.
    - DMA out tile back.
"""

import numpy as np

import concourse.bacc as bacc
import concourse.mybir as mybir
import concourse.tile as tile
from concourse.bass_utils import run_bass_kernel_spmd

N_CORES = 8
B_FULL = 64
B_PER = B_FULL // N_CORES  # 8 batches per core
N_SET = 1024
D = 512
P = 128
R = N_SET // P  # 8 set-rows per partition
F = R * D  # 4096 free elems per partition

_cache = {}


def build_nc():
    if "nc" in _cache:
        return _cache["nc"]
    f32 = mybir.dt.float32
    nc = bacc.Bacc(
        "TRN2", target_bir_lowering=False, debug=False, num_devices=N_CORES
    )
    x_d = nc.dram_tensor("x", [B_PER, N_SET, D], f32, kind="ExternalInput").ap()
    mu_d = nc.dram_tensor("mu", [1], f32, kind="ExternalInput").ap()
    gamma_d = nc.dram_tensor("gamma", [1], f32, kind="ExternalInput").ap()
    out_d = nc.dram_tensor("out", [B_PER, N_SET, D], f32, kind="ExternalOutput").ap()

    with tile.TileContext(nc) as tc:
        with (
            tc.tile_pool(name="consts", bufs=1) as consts,
            tc.tile_pool(name="xp", bufs=4) as xp,
            tc.tile_pool(name="op", bufs=4) as op,
            tc.tile_pool(name="sp", bufs=2) as sp,
            tc.tile_pool(name="ps", bufs=2, space="PSUM") as ps,
            tc.tile_pool(name="pb", bufs=2, space="PSUM") as pb,
        ):
            # ---- constants ----
            ones_col = consts.tile([P, 1], f32)  # colsum lhsT (K=128, M=1)
            nc.vector.memset(ones_col, 1.0)
            ones_row = consts.tile([1, P], f32)  # bcast lhsT template
            nc.vector.memset(ones_row, 1.0)
            mu_sb = consts.tile([1, 1], f32)
            nc.sync.dma_start(mu_sb, mu_d[None, :])
            gamma_sb = consts.tile([1, 1], f32)
            nc.sync.dma_start(gamma_sb, gamma_d[None, :])
            # gamma_row[1,128] = gamma * ones (runtime scalar from SBUF)
            gamma_row = consts.tile([1, P], f32)
            nc.vector.tensor_scalar_mul(gamma_row, ones_row, gamma_sb[:])
            # mu replicated to all 128 partitions via rank-1 PE matmul
            psum_mu = ps.tile([P, 1], f32, tag="psmu")
            nc.tensor.matmul(psum_mu, lhsT=ones_row[:], rhs=mu_sb[:], start=True, stop=True)
            mu_col = consts.tile([P, 1], f32)
            nc.vector.tensor_copy(mu_col, psum_mu)

            # ---- per-batch pipeline ----
            for b in range(B_PER):
                x_view = x_d[b].rearrange("(p r) d -> p (r d)", p=P)
                o_view = out_d[b].rearrange("(p r) d -> p (r d)", p=P)

                xt = xp.tile([P, F], f32, tag="xt")
                nc.sync.dma_start(xt, x_view)

                # colsum over all 1024 set rows -> psum_s[1, 512]
                psum_s = ps.tile([1, D], f32, tag="pss")
                for r in range(R):
                    nc.tensor.matmul(
                        psum_s,
                        lhsT=ones_col[:],
                        rhs=xt[:, r * D : (r + 1) * D],
                        start=(r == 0),
                        stop=(r == R - 1),
                    )
                s_sb = sp.tile([1, D], f32, tag="ssb")
                nc.scalar.copy(s_sb, psum_s)

                # broadcast gamma*colsum to [128, 512] via rank-1 matmul
                psum_b = pb.tile([P, D], f32, tag="psb")
                nc.tensor.matmul(
                    psum_b, lhsT=gamma_row[:], rhs=s_sb[:], start=True, stop=True
                )

                # fused: out = (x * mu) + bcast   (single DVE pass)
                ot = op.tile([P, F], f32, tag="ot")
                nc.vector.scalar_tensor_tensor(
                    out=ot[:].rearrange("p (r d) -> p r d", r=R),
                    in0=xt[:].rearrange("p (r d) -> p r d", r=R),
                    scalar=mu_col[:],
                    in1=psum_b[:, None, :].broadcast_to([P, R, D]),
                    op0=mybir.AluOpType.mult,
                    op1=mybir.AluOpType.add,
                )
                # Stores issue from the ACT HWDGE ring so a store waiting on
                # compute can't head-of-line-block the next batch's load
                # (loads use the SP ring via nc.sync).
                nc.scalar.dma_start(o_view, ot)

    nc.compile()
    _cache["nc"] = nc
    return nc


def run_spmd(x, mu, gamma, **spmd_kwargs):
    nc = build_nc()
    x = np.ascontiguousarray(x, dtype=np.float32)
    mu = np.ascontiguousarray(mu, dtype=np.float32)
    gamma = np.ascontiguousarray(gamma, dtype=np.float32)
    in_maps = [
        {"x": x[c * B_PER : (c + 1) * B_PER], "mu": mu, "gamma": gamma}
        for c in range(N_CORES)
    ]
    return run_bass_kernel_spmd(nc, in_maps, list(range(N_CORES)), **spmd_kwargs)


def kernel(x, mu, gamma):
    res = run_spmd(x, mu, gamma)
    out = np.concatenate([r["out"] for r in res.results], axis=0)
    return out


# revision 8
# speedup vs baseline: 1.2356x; 1.0710x over previous
"""ContextBasedLinear Trainium2 kernel.

Computes out = mu * x + gamma * sum(x, axis=1, keepdims=True) for
x: [64, 1024, 512] f32, mu/gamma: [1] f32.

Sharding: data-parallel on the batch dim — 8 batches per core on 8
NeuronCores; mu/gamma replicated. No cross-core comms needed.

Per-core program (x_c: [8, 1024, 512]):
  Each batch's [1024, 512] lives in SBUF as [128, 4096]: partition p
  holds set rows 8p..8p+7 (16 KB contiguous per partition), processed
  in two half-tiles [128, 2048] for pipelining.
  - colsum: PE matmuls with ones[128,1] stationary reduce the
    partition dim of each 512-wide r-slice, accumulating all 8 slices
    into one PSUM row psum_s[1, 512].
  - psum_b[128,512] = (gamma ones)[1,128].T @ s[1,512]: rank-1 matmul
    broadcasts gamma * colsum to every partition.
  - out = (x * mu) + psum_b in ONE fused DVE scalar_tensor_tensor pass
    per half, with psum_b read through a step-0 broadcast AP.
  - loads issue on the SP HWDGE ring (nc.sync), stores on the ACT ring
    (nc.scalar) so store-waits can't head-of-line-block loads.
"""

import numpy as np

import concourse.bacc as bacc
import concourse.mybir as mybir
import concourse.tile as tile
from concourse.bass_utils import run_bass_kernel_spmd

N_CORES = 8
B_FULL = 64
B_PER = B_FULL // N_CORES  # 8 batches per core
N_SET = 1024
D = 512
P = 128
R = N_SET // P  # 8 set-rows per partition
F = R * D  # 4096 free elems per partition
H = 2  # half-tiles per batch
RH = R // H  # 4 r-slices per half
FH = F // H  # 2048 free elems per half

_cache = {}


def build_nc():
    if "nc" in _cache:
        return _cache["nc"]
    f32 = mybir.dt.float32
    nc = bacc.Bacc(
        "TRN2", target_bir_lowering=False, debug=False, num_devices=N_CORES
    )
    x_d = nc.dram_tensor("x", [B_PER, N_SET, D], f32, kind="ExternalInput").ap()
    mu_d = nc.dram_tensor("mu", [1], f32, kind="ExternalInput").ap()
    gamma_d = nc.dram_tensor("gamma", [1], f32, kind="ExternalInput").ap()
    out_d = nc.dram_tensor("out", [B_PER, N_SET, D], f32, kind="ExternalOutput").ap()

    with tile.TileContext(nc) as tc:
        with (
            tc.tile_pool(name="consts", bufs=1) as consts,
            tc.tile_pool(name="xp", bufs=6) as xp,
            tc.tile_pool(name="op", bufs=6) as op,
            tc.tile_pool(name="sp", bufs=2) as sp,
            tc.tile_pool(name="ps", bufs=2, space="PSUM") as ps,
            tc.tile_pool(name="pb", bufs=2, space="PSUM") as pb,
        ):
            # ---- constants ----
            ones_col = consts.tile([P, 1], f32)  # colsum lhsT (K=128, M=1)
            nc.vector.memset(ones_col, 1.0)
            ones_row = consts.tile([1, P], f32)
            nc.vector.memset(ones_row, 1.0)
            mu_sb = consts.tile([1, 1], f32)
            nc.sync.dma_start(mu_sb, mu_d[None, :])
            gamma_sb = consts.tile([1, 1], f32)
            nc.sync.dma_start(gamma_sb, gamma_d[None, :])
            # gamma_row[1,128] = gamma * ones (runtime scalar from SBUF)
            gamma_row = consts.tile([1, P], f32)
            nc.vector.tensor_scalar_mul(gamma_row, ones_row, gamma_sb[:])
            # mu replicated to all 128 partitions via rank-1 matmul
            psum_mu = ps.tile([P, 1], f32, tag="psmu")
            nc.tensor.matmul(
                psum_mu, lhsT=ones_row[:], rhs=mu_sb[:], start=True, stop=True
            )
            mu_col = consts.tile([P, 1], f32)
            nc.vector.tensor_copy(mu_col, psum_mu)

            # ---- per-batch pipeline ----
            for b in range(B_PER):
                x_view = x_d[b].rearrange("(p r) d -> p (r d)", p=P)
                o_view = out_d[b].rearrange("(p r) d -> p (r d)", p=P)

                xts = []
                for h in range(H):
                    xt = xp.tile([P, FH], f32, tag="xt")
                    nc.sync.dma_start(xt, x_view[:, h * FH : (h + 1) * FH])
                    xts.append(xt)

                # colsum over all 1024 set rows -> psum_s[1, 512]
                psum_s = ps.tile([1, D], f32, tag="pss")
                for h in range(H):
                    for j in range(RH):
                        nc.tensor.matmul(
                            psum_s,
                            lhsT=ones_col[:],
                            rhs=xts[h][:, j * D : (j + 1) * D],
                            start=(h == 0 and j == 0),
                            stop=(h == H - 1 and j == RH - 1),
                        )
                s_sb = sp.tile([1, D], f32, tag="ssb")
                nc.scalar.copy(s_sb, psum_s)

                # broadcast gamma*colsum to [128, 512] via rank-1 matmul
                psum_b = pb.tile([P, D], f32, tag="psb")
                nc.tensor.matmul(
                    psum_b, lhsT=gamma_row[:], rhs=s_sb[:], start=True, stop=True
                )

                # fused: out = (x * mu) + bcast   (single DVE pass per half)
                for h in range(H):
                    ot = op.tile([P, FH], f32, tag="ot")
                    nc.vector.scalar_tensor_tensor(
                        out=ot[:].rearrange("p (r d) -> p r d", r=RH),
                        in0=xts[h][:].rearrange("p (r d) -> p r d", r=RH),
                        scalar=mu_col[:],
                        in1=psum_b[:, None, :].broadcast_to([P, RH, D]),
                        op0=mybir.AluOpType.mult,
                        op1=mybir.AluOpType.add,
                    )
                    nc.scalar.dma_start(o_view[:, h * FH : (h + 1) * FH], ot)

    nc.compile()
    _cache["nc"] = nc
    return nc


def run_spmd(x, mu, gamma, **spmd_kwargs):
    nc = build_nc()
    x = np.ascontiguousarray(x, dtype=np.float32)
    mu = np.ascontiguousarray(mu, dtype=np.float32)
    gamma = np.ascontiguousarray(gamma, dtype=np.float32)
    in_maps = [
        {"x": x[c * B_PER : (c + 1) * B_PER], "mu": mu, "gamma": gamma}
        for c in range(N_CORES)
    ]
    return run_bass_kernel_spmd(nc, in_maps, list(range(N_CORES)), **spmd_kwargs)


def kernel(x, mu, gamma):
    res = run_spmd(x, mu, gamma)
    out = np.concatenate([r["out"] for r in res.results], axis=0)
    return out
